# revision 12
# baseline (speedup 1.0000x reference)
"""Multi-head dot-product attention (B=2, S=2048, D=2048, H=16, HD=128) with
RoPE + causal mask, sharded over 8 NeuronCores: batch (2) x head-groups (4).

Each core computes 4 heads of one batch element end-to-end (QKV projections,
RoPE, causal softmax attention, output projection); the host sums the two
head-group partials per batch element.

Self-contained: hardcodes all shapes; builds/compiles the Bass program once
per process and runs it via run_bass_kernel_spmd on cores 0-7.
"""

import os
import sys
import types

import ml_dtypes
import numpy as np

B, S, D, H, HD = 2, 2048, 2048, 16, 128
HPC = 4                 # heads per core
HW = HPC * HD           # 512: per-core projection width
NQB = S // 512          # 4 query blocks / token quarters of 512
NKT = S // 128          # 16 key-token tiles of 128
NDC = D // 128          # 16 contraction chunks of 128
N_CORES = 8
SCALE = float(HD) ** -0.5

BF16 = ml_dtypes.bfloat16

_CACHE = {}


def _install_ntff_hook():
    """The image's antenv lacks axon_hooks, so boot() couldn't register the
    NTFF profile hook; recreate the module + hook so trace=True works."""
    if "antenv.axon_hooks" in sys.modules:
        return
    try:
        import antenv  # noqa: F401
        mod = types.ModuleType("antenv.axon_hooks")
        _h = [None]
        mod.set_axon_ntff_profile_hook = lambda h: _h.__setitem__(0, h)
        mod.get_axon_ntff_profile_hook = lambda: _h[0]
        sys.modules["antenv.axon_hooks"] = mod
        from trn_agent_boot.trn_boot import _ntff_profile_via_ctypes
        mod.set_axon_ntff_profile_hook(
            _ntff_profile_via_ctypes("/opt/axon/libaxon_pjrt.so"))
    except Exception:
        pass


def _build():
    import concourse.mybir as mybir
    import concourse.tile as tile
    from concourse import bacc
    from concourse import bass_isa

    f32 = mybir.dt.float32
    bf16 = mybir.dt.bfloat16
    fp16 = mybir.dt.float16
    Exp = mybir.ActivationFunctionType.Exp

    nc = bacc.Bacc("TRN2", target_bir_lowering=False, debug=False,
                   enable_asserts=True, num_devices=N_CORES)

    dram = {}
    for name, shape, dt in [
        ("xqT", [D, S], bf16), ("xkvT", [D, S], bf16),
        ("wq", [D, HW], bf16), ("wk", [D, HW], bf16), ("wv", [D, HW], bf16),
        ("wo", [HW, D], bf16),
        ("sinT", [HD, S], bf16), ("cosT", [HD, S], bf16),
        ("rmatT", [HD, HD], bf16),
        ("ones_col", [128, 1], fp16), ("ones_row", [1, 128], fp16),
        ("maskt", [128, 4 * 512], fp16),
    ]:
        dram[name] = nc.dram_tensor(name, shape, dt, kind="ExternalInput").ap()
    outp = nc.dram_tensor("outp", [S, D], f32, kind="ExternalOutput").ap()

    with tile.TileContext(nc) as tc:
        with (
            tc.tile_pool(name="const", bufs=1) as cpool,
            tc.tile_pool(name="kt", bufs=1) as kt_pool,
            tc.tile_pool(name="qt", bufs=1) as qt_pool,
            tc.tile_pool(name="vsb", bufs=1) as v_pool,
            tc.tile_pool(name="ctxn", bufs=1) as ctx_pool,
            tc.tile_pool(name="wkv", bufs=1) as wkv_pool,
            tc.tile_pool(name="xin", bufs=2) as xpool,
            tc.tile_pool(name="raw", bufs=2) as raw_pool,
            tc.tile_pool(name="t12", bufs=4) as t12_pool,
            tc.tile_pool(name="pp", bufs=8) as ppool,
            tc.tile_pool(name="sacc", bufs=6) as sacc_pool,
            tc.tile_pool(name="rs", bufs=2) as rs_pool,
            tc.tile_pool(name="rcp", bufs=2) as rpool,
            tc.tile_pool(name="osb", bufs=4) as opool,
            # one PSUM pool for the whole kernel: 4 tags x 2 bufs = 8 banks;
            # no pool-boundary stalls between phases
            tc.tile_pool(name="ps", space="PSUM", bufs=2) as pspool,
        ):
            def load_chunks(pool, name, nch, width, tag=None, eng=None):
                # [nch*128, width] dram -> one [128, nch*width] sbuf tile
                t = pool.tile([128, nch * width], bf16, tag=tag or name,
                              name=name + "_sb")
                for i in range(nch):
                    (eng or nc.sync).dma_start(
                        t[:, i * width:(i + 1) * width],
                        dram[name][i * 128:(i + 1) * 128, :])
                return t

            def load(name, shape, dt=bf16):
                t = cpool.tile(shape, dt, tag=name, name=name)
                nc.scalar.dma_start(t[:], dram[name][:])
                return t

            # startup: three DMA queues in parallel so the first projection
            # matmuls start ASAP: wk on Scalar, xkvT q0 on Sync, wv on GpSimd
            wk_sb = load_chunks(wkv_pool, "wk", NDC, HW, eng=nc.scalar)
            sinT = load("sinT", [HD, S])
            cosT = load("cosT", [HD, S])
            rmatT = load("rmatT", [HD, HD])
            wv_sb = load_chunks(wkv_pool, "wv", NDC, HW, eng=nc.scalar)
            maskt = load("maskt", [128, 4 * 512], fp16)
            ones_col = load("ones_col", [128, 1], fp16)
            ones_row = load("ones_row", [1, 128], fp16)
            wo_sb = load_chunks(cpool, "wo", HW // 128, D, eng=nc.scalar)

            # per-head projection outputs (+rope for Q/K)
            kt_sb = [kt_pool.tile([128, S], bf16, tag=f"kt{h}", name=f"kt{h}")
                     for h in range(HPC)]
            qt_sb = [qt_pool.tile([128, S], bf16, tag=f"qt{h}", name=f"qt{h}")
                     for h in range(HPC)]
            v_sb = v_pool.tile([128, NKT * HW], fp16, tag="v", name="v_sb")
            ctx_sb = [ctx_pool.tile([128, S], bf16, tag=f"ctx{h}",
                                    name=f"ctx{h}") for h in range(HPC)]

            def proj_quarter(xname, tq, w_sb, out_tiles, with_v=False):
                """Token-quarter tq of x^T: per-head 512-wide projection
                (+rope) into out_tiles[h][:, tq*512:...]; for the kv pass
                also the 4 V token-tiles of this quarter."""
                sl = slice(tq * 512, (tq + 1) * 512)
                xt = xpool.tile([128, NDC * 512], bf16, tag="xin",
                                name=f"{xname}_{tq}")
                for kc in range(NDC):
                    eng = nc.sync if kc % 2 == 0 else nc.gpsimd
                    eng.dma_start(
                        xt[:, kc * 512:(kc + 1) * 512],
                        dram[xname][kc * 128:(kc + 1) * 128, sl])
                for h in range(HPC):
                    ps = pspool.tile([128, 512], f32, tag="A",
                                     name=f"ps_{xname}_{tq}_{h}")
                    for kc in range(NDC):
                        nc.tensor.matmul(
                            ps[:],
                            lhsT=w_sb[:, kc * HW + h * HD:
                                      kc * HW + (h + 1) * HD],
                            rhs=xt[:, kc * 512:(kc + 1) * 512],
                            start=(kc == 0), stop=(kc == NDC - 1))
                    raw = raw_pool.tile([128, 512], bf16, tag="raw",
                                        name=f"raw_{xname}_{tq}_{h}")
                    nc.scalar.copy(raw[:], ps[:])
                    # rope: out = raw*cos + (R @ raw)*sin
                    rot = pspool.tile([128, 512], f32, tag="B",
                                      name=f"rot_{xname}_{tq}_{h}")
                    nc.tensor.matmul(rot[:], lhsT=rmatT[:], rhs=raw[:])
                    t1 = t12_pool.tile([128, 512], bf16, tag="t1", name="t1")
                    nc.vector.tensor_mul(t1[:], rot[:], sinT[:, sl])
                    t2 = t12_pool.tile([128, 512], bf16, tag="t2", name="t2")
                    nc.vector.tensor_mul(t2[:], raw[:], cosT[:, sl])
                    nc.vector.tensor_add(out_tiles[h][:, sl], t1[:], t2[:])
                if with_v:
                    for ti in range(4):
                        t = tq * 4 + ti
                        ps = pspool.tile([128, 512], f32, tag="C",
                                         name=f"vps_{t}")
                        for kc in range(NDC):
                            nc.tensor.matmul(
                                ps[:],
                                lhsT=xt[:, kc * 512 + ti * 128:
                                        kc * 512 + (ti + 1) * 128],
                                rhs=wv_sb[:, kc * HW:(kc + 1) * HW],
                                start=(kc == 0), stop=(kc == NDC - 1))
                        nc.scalar.copy(v_sb[:, t * HW:(t + 1) * HW], ps[:])

            # ---- phases 1-2: K^T + V (stream xkvT), then Q^T (stream xqT)
            for tq in range(NQB):
                proj_quarter("xkvT", tq, wk_sb, kt_sb, with_v=True)
            wq_sb = load_chunks(wkv_pool, "wq", NDC, HW, tag="wk")
            for tq in range(NQB):
                proj_quarter("xqT", tq, wq_sb, qt_sb)

            # ---- phases 3+4: attention with interleaved output projection.
            # All 4 heads' kt-streams interleave inside a query block so the
            # softmax-normalization chains and PSUM slot recycling hide behind
            # the other heads' matmul/exp work.
            for qb in range(NQB):
                qsl = slice(qb * 512, (qb + 1) * 512)
                last = 4 * qb + 3
                ctx_ps = [pspool.tile([128, 512], f32,
                                      tag=("A" if h < 2 else "B"),
                                      name=f"ctxps_{h}_{qb}")
                          for h in range(HPC)]
                accs = [sacc_pool.tile([128, 512], fp16, tag="acc",
                                       name=f"acc_{h}_{qb}")
                        for h in range(HPC)]
                for kt in range(last + 1):
                    for h in range(HPC):
                        st = pspool.tile([128, 512], f32, tag="C",
                                         name=f"st_{h}_{qb}_{kt}")
                        nc.tensor.matmul(
                            st[:],
                            lhsT=kt_sb[h][:, kt * 128:(kt + 1) * 128],
                            rhs=qt_sb[h][:, qsl])
                        p = ppool.tile([128, 512], fp16, tag="p",
                                       name=f"p_{h}_{qb}_{kt}")
                        nc.scalar.activation(p[:], st[:], Exp, scale=SCALE)
                        if kt >= 4 * qb:
                            v = kt - 4 * qb
                            nc.vector.tensor_mul(
                                p[:], p[:], maskt[:, v * 512:(v + 1) * 512])
                        if kt == 0:
                            nc.vector.tensor_copy(accs[h][:], p[:])
                        else:
                            nc.vector.tensor_add(accs[h][:], accs[h][:], p[:])
                        nc.tensor.matmul(
                            ctx_ps[h][:],
                            lhsT=v_sb[:, kt * HW + h * HD:
                                      kt * HW + (h + 1) * HD],
                            rhs=p[:], start=(kt == 0), stop=(kt == last))
                for h in range(HPC):
                    s_ps = pspool.tile([1, 512], f32, tag="D",
                                       name=f"sps_{h}_{qb}")
                    nc.tensor.matmul(s_ps[:], lhsT=ones_col[:], rhs=accs[h][:])
                    s_sb = rs_pool.tile([1, 512], fp16, tag="ssb",
                                        name=f"ssb_{h}_{qb}")
                    nc.scalar.copy(s_sb[:], s_ps[:])
                    rb_ps = pspool.tile([128, 512], f32, tag="D",
                                        name=f"rbps_{h}_{qb}")
                    nc.tensor.matmul(rb_ps[:], lhsT=ones_row[:], rhs=s_sb[:])
                    rb_sb = rpool.tile([128, 512], f32, tag="rb",
                                       name=f"rbsb_{h}_{qb}")
                    nc.vector.reciprocal_approx_fast(rb_sb[:], rb_ps[:])
                    nc.vector.tensor_mul(ctx_sb[h][:, qsl], ctx_ps[h][:],
                                         rb_sb[:])
                # output projection for this query block
                for qt in range(qb * 4, qb * 4 + 4):
                    for db in range(NQB):
                        ps = pspool.tile([128, 512], f32, tag="D",
                                         name=f"ops_{qt}_{db}")
                        for h in range(HPC):
                            nc.tensor.matmul(
                                ps[:],
                                lhsT=ctx_sb[h][:, qt * 128:(qt + 1) * 128],
                                rhs=wo_sb[:, h * D + db * 512:
                                          h * D + (db + 1) * 512],
                                start=(h == 0), stop=(h == HPC - 1))
                        osb = opool.tile([128, 512], f32, tag="o",
                                         name=f"osb_{qt}_{db}")
                        nc.vector.tensor_copy(osb[:], ps[:])
                        nc.sync.dma_start(
                            outp[qt * 128:(qt + 1) * 128,
                                 db * 512:(db + 1) * 512], osb[:])

    nc.compile()
    return nc


def _host_constants():
    # sin/cos tables exactly as the flaxformer reference (fp32 math)
    fraction = np.arange(0, HD, 2, dtype=np.float32) / np.float32(HD)
    timescale = (np.float32(10000.0) ** fraction).astype(np.float32)
    sinusoid = np.einsum(
        "i,j->ij", np.arange(S, dtype=np.float32),
        (np.float32(1.0) / timescale)).astype(np.float32)
    sinusoid = np.concatenate([sinusoid, sinusoid], axis=-1)  # [S, HD]
    sinT = np.sin(sinusoid).astype(np.float32).T.copy()
    cosT = np.cos(sinusoid).astype(np.float32).T.copy()

    # rotate_half as a matmul: rot = R @ x, lhsT = R^T
    R = np.zeros((HD, HD), np.float32)
    for i in range(64):
        R[i, i + 64] = -1.0
        R[i + 64, i] = 1.0

    # causal mask variants for the 4 diagonal sub-blocks: allowed iff
    # q - k >= 0 with q = 512*qb + c, k = 128*(4*qb + v) + r
    r = np.arange(128)[:, None]
    c = np.arange(512)[None, :]
    maskt = np.concatenate(
        [(c - r >= 128 * v).astype(np.float32) for v in range(4)], axis=1)

    return {
        "sinT": sinT.astype(BF16), "cosT": cosT.astype(BF16),
        "rmatT": R.T.copy().astype(BF16),
        "ones_col": np.ones((128, 1), np.float16),
        "ones_row": np.ones((1, 128), np.float16),
        "maskt": maskt.astype(np.float16),
    }


def kernel(inputs_q, inputs_kv, wq, wk, wv, wo, mask=None):
    _install_ntff_hook()
    from concourse import bass_utils

    if "nc" not in _CACHE:
        _CACHE["nc"] = _build()
        _CACHE["consts"] = _host_constants()
    nc = _CACHE["nc"]
    consts = _CACHE["consts"]

    wq2 = np.asarray(wq, np.float32).reshape(D, H * HD)
    wk2 = np.asarray(wk, np.float32).reshape(D, H * HD)
    wv2 = np.asarray(wv, np.float32).reshape(D, H * HD)
    wo2 = np.asarray(wo, np.float32).reshape(H * HD, D)
    xq = np.asarray(inputs_q, np.float32)
    xkv = np.asarray(inputs_kv, np.float32)

    in_maps = []
    for c in range(N_CORES):
        b, hg = divmod(c, H // HPC)
        hs = slice(hg * HW, (hg + 1) * HW)
        in_maps.append({
            "xqT": np.ascontiguousarray(xq[b].T).astype(BF16),
            "xkvT": np.ascontiguousarray(xkv[b].T).astype(BF16),
            "wq": wq2[:, hs].astype(BF16),
            "wk": wk2[:, hs].astype(BF16),
            "wv": wv2[:, hs].astype(BF16),
            "wo": wo2[hs, :].astype(BF16),
            **consts,
        })

    trace = bool(int(os.environ.get("KERNEL_TRACE", "0")))
    res = bass_utils.run_bass_kernel_spmd(
        nc, in_maps, core_ids=list(range(N_CORES)), trace=trace)
    _CACHE["last_result"] = res

    out = np.zeros((B, S, D), np.float32)
    for c in range(N_CORES):
        out[c // (H // HPC)] += res.results[c]["outp"]
    return out


# revision 13
# speedup vs baseline: 1.0068x; 1.0068x over previous
"""Multi-head dot-product attention (B=2, S=2048, D=2048, H=16, HD=128) with
RoPE + causal mask, sharded over 8 NeuronCores: batch (2) x head-groups (4).

Each core computes 4 heads of one batch element end-to-end (QKV projections,
RoPE, causal softmax attention, output projection); the host sums the two
head-group partials per batch element.

Self-contained: hardcodes all shapes; builds/compiles the Bass program once
per process and runs it via run_bass_kernel_spmd on cores 0-7.
"""

import os
import sys
import types

import ml_dtypes
import numpy as np

B, S, D, H, HD = 2, 2048, 2048, 16, 128
HPC = 4                 # heads per core
HW = HPC * HD           # 512: per-core projection width
NQB = S // 512          # 4 query blocks / token quarters of 512
NKT = S // 128          # 16 key-token tiles of 128
NDC = D // 128          # 16 contraction chunks of 128
N_CORES = 8
SCALE = float(HD) ** -0.5

BF16 = ml_dtypes.bfloat16

_CACHE = {}


def _install_ntff_hook():
    """The image's antenv lacks axon_hooks, so boot() couldn't register the
    NTFF profile hook; recreate the module + hook so trace=True works."""
    if "antenv.axon_hooks" in sys.modules:
        return
    try:
        import antenv  # noqa: F401
        mod = types.ModuleType("antenv.axon_hooks")
        _h = [None]
        mod.set_axon_ntff_profile_hook = lambda h: _h.__setitem__(0, h)
        mod.get_axon_ntff_profile_hook = lambda: _h[0]
        sys.modules["antenv.axon_hooks"] = mod
        from trn_agent_boot.trn_boot import _ntff_profile_via_ctypes
        mod.set_axon_ntff_profile_hook(
            _ntff_profile_via_ctypes("/opt/axon/libaxon_pjrt.so"))
    except Exception:
        pass


def _build():
    import concourse.mybir as mybir
    import concourse.tile as tile
    from concourse import bacc
    from concourse import bass_isa

    f32 = mybir.dt.float32
    bf16 = mybir.dt.bfloat16
    fp16 = mybir.dt.float16
    Exp = mybir.ActivationFunctionType.Exp

    nc = bacc.Bacc("TRN2", target_bir_lowering=False, debug=False,
                   enable_asserts=True, num_devices=N_CORES)

    dram = {}
    for name, shape, dt in [
        ("xqT", [D, S], bf16), ("xkvT", [D, S], bf16),
        ("wq", [D, HW], bf16), ("wk", [D, HW], bf16), ("wv", [D, HW], bf16),
        ("wo", [HW, D], bf16),
        ("sinT", [HD, S], bf16), ("cosT", [HD, S], bf16),
        ("rmatT", [HD, HD], bf16),
        ("ones_col", [128, 1], fp16), ("ones_row", [1, 128], fp16),
        ("maskt", [128, 4 * 512], fp16),
    ]:
        dram[name] = nc.dram_tensor(name, shape, dt, kind="ExternalInput").ap()
    outp = nc.dram_tensor("outp", [S, D], f32, kind="ExternalOutput").ap()

    with tile.TileContext(nc) as tc:
        with (
            tc.tile_pool(name="const", bufs=1) as cpool,
            tc.tile_pool(name="kt", bufs=1) as kt_pool,
            tc.tile_pool(name="qt", bufs=1) as qt_pool,
            tc.tile_pool(name="vsb", bufs=1) as v_pool,
            tc.tile_pool(name="ctxn", bufs=1) as ctx_pool,
            tc.tile_pool(name="wkv", bufs=1) as wkv_pool,
            tc.tile_pool(name="xin", bufs=2) as xpool,
            tc.tile_pool(name="raw", bufs=2) as raw_pool,
            tc.tile_pool(name="t12", bufs=4) as t12_pool,
            tc.tile_pool(name="pp", bufs=8) as ppool,
            tc.tile_pool(name="sacc", bufs=6) as sacc_pool,
            tc.tile_pool(name="rs", bufs=2) as rs_pool,
            tc.tile_pool(name="rcp", bufs=2) as rpool,
            tc.tile_pool(name="osb", bufs=4) as opool,
            # one PSUM pool for the whole kernel: 4 tags x 2 bufs = 8 banks;
            # no pool-boundary stalls between phases
            tc.tile_pool(name="ps", space="PSUM", bufs=2) as pspool,
        ):
            def load_chunks(pool, name, nch, width, tag=None, eng=None):
                # [nch*128, width] dram -> one [128, nch*width] sbuf tile
                t = pool.tile([128, nch * width], bf16, tag=tag or name,
                              name=name + "_sb")
                for i in range(nch):
                    (eng or nc.sync).dma_start(
                        t[:, i * width:(i + 1) * width],
                        dram[name][i * 128:(i + 1) * 128, :])
                return t

            def load(name, shape, dt=bf16):
                t = cpool.tile(shape, dt, tag=name, name=name)
                nc.scalar.dma_start(t[:], dram[name][:])
                return t

            # startup: three DMA queues in parallel so the first projection
            # matmuls start ASAP: wk on Scalar, xkvT q0 on Sync, wv on GpSimd
            wk_sb = load_chunks(wkv_pool, "wk", NDC, HW, eng=nc.scalar)
            sinT = load("sinT", [HD, S])
            cosT = load("cosT", [HD, S])
            rmatT = load("rmatT", [HD, HD])
            wv_sb = load_chunks(wkv_pool, "wv", NDC, HW, eng=nc.scalar)
            maskt = load("maskt", [128, 4 * 512], fp16)
            ones_col = load("ones_col", [128, 1], fp16)
            ones_row = load("ones_row", [1, 128], fp16)
            wo_sb = load_chunks(cpool, "wo", HW // 128, D, eng=nc.scalar)

            # per-head projection outputs (+rope for Q/K)
            kt_sb = [kt_pool.tile([128, S], bf16, tag=f"kt{h}", name=f"kt{h}")
                     for h in range(HPC)]
            qt_sb = [qt_pool.tile([128, S], bf16, tag=f"qt{h}", name=f"qt{h}")
                     for h in range(HPC)]
            v_sb = v_pool.tile([128, NKT * HW], fp16, tag="v", name="v_sb")
            ctx_sb = [ctx_pool.tile([128, S], bf16, tag=f"ctx{h}",
                                    name=f"ctx{h}") for h in range(HPC)]

            def proj_quarter(xname, tq, w_sb, out_tiles, with_v=False):
                """Token-quarter tq of x^T: per-head 512-wide projection
                (+rope) into out_tiles[h][:, tq*512:...]; for the kv pass
                also the 4 V token-tiles of this quarter."""
                sl = slice(tq * 512, (tq + 1) * 512)
                xt = xpool.tile([128, NDC * 512], bf16, tag="xin",
                                name=f"{xname}_{tq}")
                for kc in range(NDC):
                    nc.sync.dma_start(
                        xt[:, kc * 512:(kc + 1) * 512],
                        dram[xname][kc * 128:(kc + 1) * 128, sl])
                for h in range(HPC):
                    ps = pspool.tile([128, 512], f32, tag="A",
                                     name=f"ps_{xname}_{tq}_{h}")
                    for kc in range(NDC):
                        nc.tensor.matmul(
                            ps[:],
                            lhsT=w_sb[:, kc * HW + h * HD:
                                      kc * HW + (h + 1) * HD],
                            rhs=xt[:, kc * 512:(kc + 1) * 512],
                            start=(kc == 0), stop=(kc == NDC - 1))
                    raw = raw_pool.tile([128, 512], bf16, tag="raw",
                                        name=f"raw_{xname}_{tq}_{h}")
                    nc.scalar.copy(raw[:], ps[:])
                    # rope: out = raw*cos + (R @ raw)*sin
                    rot = pspool.tile([128, 512], f32, tag="B",
                                      name=f"rot_{xname}_{tq}_{h}")
                    nc.tensor.matmul(rot[:], lhsT=rmatT[:], rhs=raw[:])
                    t1 = t12_pool.tile([128, 512], bf16, tag="t1", name="t1")
                    nc.vector.tensor_mul(t1[:], rot[:], sinT[:, sl])
                    t2 = t12_pool.tile([128, 512], bf16, tag="t2", name="t2")
                    nc.vector.tensor_mul(t2[:], raw[:], cosT[:, sl])
                    nc.vector.tensor_add(out_tiles[h][:, sl], t1[:], t2[:])
                if with_v:
                    for ti in range(4):
                        t = tq * 4 + ti
                        ps = pspool.tile([128, 512], f32, tag="C",
                                         name=f"vps_{t}")
                        for kc in range(NDC):
                            nc.tensor.matmul(
                                ps[:],
                                lhsT=xt[:, kc * 512 + ti * 128:
                                        kc * 512 + (ti + 1) * 128],
                                rhs=wv_sb[:, kc * HW:(kc + 1) * HW],
                                start=(kc == 0), stop=(kc == NDC - 1))
                        nc.scalar.copy(v_sb[:, t * HW:(t + 1) * HW], ps[:])

            # ---- phases 1-2: K^T + V (stream xkvT), then Q^T (stream xqT)
            for tq in range(NQB):
                proj_quarter("xkvT", tq, wk_sb, kt_sb, with_v=True)
            wq_sb = load_chunks(wkv_pool, "wq", NDC, HW, tag="wk")
            for tq in range(NQB):
                proj_quarter("xqT", tq, wq_sb, qt_sb)

            # ---- phases 3+4: attention with interleaved output projection.
            # All 4 heads' kt-streams interleave inside a query block so the
            # softmax-normalization chains and PSUM slot recycling hide behind
            # the other heads' matmul/exp work.
            for qb in range(NQB):
                qsl = slice(qb * 512, (qb + 1) * 512)
                last = 4 * qb + 3
                ctx_ps = [pspool.tile([128, 512], f32,
                                      tag=("A" if h < 2 else "B"),
                                      name=f"ctxps_{h}_{qb}")
                          for h in range(HPC)]
                accs = [sacc_pool.tile([128, 512], fp16, tag="acc",
                                       name=f"acc_{h}_{qb}")
                        for h in range(HPC)]
                for kt in range(last + 1):
                    for h in range(HPC):
                        st = pspool.tile([128, 512], f32, tag="C",
                                         name=f"st_{h}_{qb}_{kt}")
                        nc.tensor.matmul(
                            st[:],
                            lhsT=kt_sb[h][:, kt * 128:(kt + 1) * 128],
                            rhs=qt_sb[h][:, qsl])
                        p = ppool.tile([128, 512], fp16, tag="p",
                                       name=f"p_{h}_{qb}_{kt}")
                        nc.scalar.activation(p[:], st[:], Exp, scale=SCALE)
                        if kt >= 4 * qb:
                            v = kt - 4 * qb
                            nc.vector.tensor_mul(
                                p[:], p[:], maskt[:, v * 512:(v + 1) * 512])
                        if kt == 0:
                            nc.vector.tensor_copy(accs[h][:], p[:])
                        else:
                            nc.vector.tensor_add(accs[h][:], accs[h][:], p[:])
                        nc.tensor.matmul(
                            ctx_ps[h][:],
                            lhsT=v_sb[:, kt * HW + h * HD:
                                      kt * HW + (h + 1) * HD],
                            rhs=p[:], start=(kt == 0), stop=(kt == last))
                for h in range(HPC):
                    s_ps = pspool.tile([1, 512], f32, tag="D",
                                       name=f"sps_{h}_{qb}")
                    nc.tensor.matmul(s_ps[:], lhsT=ones_col[:], rhs=accs[h][:])
                    s_sb = rs_pool.tile([1, 512], fp16, tag="ssb",
                                        name=f"ssb_{h}_{qb}")
                    nc.scalar.copy(s_sb[:], s_ps[:])
                    rb_ps = pspool.tile([128, 512], f32, tag="D",
                                        name=f"rbps_{h}_{qb}")
                    nc.tensor.matmul(rb_ps[:], lhsT=ones_row[:], rhs=s_sb[:])
                    rb_sb = rpool.tile([128, 512], f32, tag="rb",
                                       name=f"rbsb_{h}_{qb}")
                    nc.vector.reciprocal_approx_fast(rb_sb[:], rb_ps[:])
                    nc.vector.tensor_mul(ctx_sb[h][:, qsl], ctx_ps[h][:],
                                         rb_sb[:])
                # output projection for this query block
                for qt in range(qb * 4, qb * 4 + 4):
                    for db in range(NQB):
                        ps = pspool.tile([128, 512], f32, tag="D",
                                         name=f"ops_{qt}_{db}")
                        for h in range(HPC):
                            nc.tensor.matmul(
                                ps[:],
                                lhsT=ctx_sb[h][:, qt * 128:(qt + 1) * 128],
                                rhs=wo_sb[:, h * D + db * 512:
                                          h * D + (db + 1) * 512],
                                start=(h == 0), stop=(h == HPC - 1))
                        osb = opool.tile([128, 512], f32, tag="o",
                                         name=f"osb_{qt}_{db}")
                        nc.vector.tensor_copy(osb[:], ps[:])
                        nc.sync.dma_start(
                            outp[qt * 128:(qt + 1) * 128,
                                 db * 512:(db + 1) * 512], osb[:])

    nc.compile()
    return nc


def _host_constants():
    # sin/cos tables exactly as the flaxformer reference (fp32 math)
    fraction = np.arange(0, HD, 2, dtype=np.float32) / np.float32(HD)
    timescale = (np.float32(10000.0) ** fraction).astype(np.float32)
    sinusoid = np.einsum(
        "i,j->ij", np.arange(S, dtype=np.float32),
        (np.float32(1.0) / timescale)).astype(np.float32)
    sinusoid = np.concatenate([sinusoid, sinusoid], axis=-1)  # [S, HD]
    sinT = np.sin(sinusoid).astype(np.float32).T.copy()
    cosT = np.cos(sinusoid).astype(np.float32).T.copy()

    # rotate_half as a matmul: rot = R @ x, lhsT = R^T
    R = np.zeros((HD, HD), np.float32)
    for i in range(64):
        R[i, i + 64] = -1.0
        R[i + 64, i] = 1.0

    # causal mask variants for the 4 diagonal sub-blocks: allowed iff
    # q - k >= 0 with q = 512*qb + c, k = 128*(4*qb + v) + r
    r = np.arange(128)[:, None]
    c = np.arange(512)[None, :]
    maskt = np.concatenate(
        [(c - r >= 128 * v).astype(np.float32) for v in range(4)], axis=1)

    return {
        "sinT": sinT.astype(BF16), "cosT": cosT.astype(BF16),
        "rmatT": R.T.copy().astype(BF16),
        "ones_col": np.ones((128, 1), np.float16),
        "ones_row": np.ones((1, 128), np.float16),
        "maskt": maskt.astype(np.float16),
    }


def kernel(inputs_q, inputs_kv, wq, wk, wv, wo, mask=None):
    _install_ntff_hook()
    from concourse import bass_utils

    if "nc" not in _CACHE:
        _CACHE["nc"] = _build()
        _CACHE["consts"] = _host_constants()
    nc = _CACHE["nc"]
    consts = _CACHE["consts"]

    wq2 = np.asarray(wq, np.float32).reshape(D, H * HD)
    wk2 = np.asarray(wk, np.float32).reshape(D, H * HD)
    wv2 = np.asarray(wv, np.float32).reshape(D, H * HD)
    wo2 = np.asarray(wo, np.float32).reshape(H * HD, D)
    xq = np.asarray(inputs_q, np.float32)
    xkv = np.asarray(inputs_kv, np.float32)

    in_maps = []
    for c in range(N_CORES):
        b, hg = divmod(c, H // HPC)
        hs = slice(hg * HW, (hg + 1) * HW)
        in_maps.append({
            "xqT": np.ascontiguousarray(xq[b].T).astype(BF16),
            "xkvT": np.ascontiguousarray(xkv[b].T).astype(BF16),
            "wq": wq2[:, hs].astype(BF16),
            "wk": wk2[:, hs].astype(BF16),
            "wv": wv2[:, hs].astype(BF16),
            "wo": wo2[hs, :].astype(BF16),
            **consts,
        })

    trace = bool(int(os.environ.get("KERNEL_TRACE", "0")))
    res = bass_utils.run_bass_kernel_spmd(
        nc, in_maps, core_ids=list(range(N_CORES)), trace=trace)
    _CACHE["last_result"] = res

    out = np.zeros((B, S, D), np.float32)
    for c in range(N_CORES):
        out[c // (H // HPC)] += res.results[c]["outp"]
    return out
